# revision 1
# baseline (speedup 1.0000x reference)
"""Trainium2 Bass kernel for the GAWA decoder (2-layer GRU + degenerate
single-position cross-attention + vocab projection), data-parallel over 8
NeuronCores.

Key algebraic simplifications (exact, not approximations):
  * softmax over a length-1 axis is identically 1, so the whole attention
    q/score path collapses; logits(t) = h1(t) @ proj_w.T + c where
    c = (ev @ attn_out_w.T + attn_out_b) @ proj_w.T + proj_b is a per-batch
    constant.
  * layer-0 input gates split as ce(t) @ w_ce.T + (eword @ w_e.T + biases),
    the second term being a per-batch constant g_e folded into PSUM via an
    identity matmul each step.

Layout: everything transposed — hidden/gate dims on SBUF partitions, the
512-row per-core batch on the free dim, so the GRU recurrence needs no
per-step transposes.  fp16 matmul operands (PE streams fp16 at the same
per-column rate as fp32; fp32 accumulation in PSUM), fp16 elementwise for
DVE 2x mode, fp32 logits output.
"""

import os
import sys

for _p in ("/opt/trn_rl_repo", "/root/.axon_site/_ro/trn_rl_repo"):
    if os.path.isdir(_p) and _p not in sys.path:
        sys.path.insert(0, _p)

import numpy as np

import concourse.bacc as bacc
import concourse.mybir as mybir
import concourse.tile as tile
from concourse.bass_utils import run_bass_kernel_spmd

B, T, V = 4096, 32, 256
E, CE, H = 768, 64, 256
NCORES = 8
BP = B // NCORES  # 512 batch rows per core
BOS, PAD = 1, 0

F16 = mybir.dt.float16
F32 = mybir.dt.float32
AF = mybir.ActivationFunctionType

# bias table column layout (each col is one 128-partition slice)
_BC_GE = 0      # 6 cols: b_ih0 + b_hh0 (r,z only) per gate row-tile
_BC_H0 = 6      # 2 cols: eword_proj_b
_BC_VAL = 8     # 2 cols: val_b
_BC_BV = 10     # 2 cols: attn_in_b v-part
_BC_AO = 12     # 2 cols: attn_out_b
_BC_HN0 = 14    # 2 cols: b_hh0 n-part
_BC_RZ1 = 16    # 4 cols: (b_ih1 + b_hh1) r,z
_BC_HN1 = 20    # 2 cols: b_hh1 n-part
_BC_IN1 = 22    # 2 cols: b_ih1 n-part
_NBC = 24

_CACHE = {}


def _build_nc():
    nc = bacc.Bacc("TRN2", target_bir_lowering=False, debug=False,
                   num_devices=NCORES)

    dt = nc.dram_tensor
    ewordT = dt("ewordT", [E, BP], F16, kind="ExternalInput")
    ceT = dt("ceT", [CE, T, BP], F16, kind="ExternalInput")
    whh0T = dt("whh0T", [H, 3 * H], F16, kind="ExternalInput")
    wceT = dt("wceT", [CE, 3 * H], F16, kind="ExternalInput")
    wih1T = dt("wih1T", [H, 3 * H], F16, kind="ExternalInput")
    whh1T = dt("whh1T", [H, 3 * H], F16, kind="ExternalInput")
    projT = dt("projT", [H, V], F16, kind="ExternalInput")
    weT = dt("weT", [E, 3 * H], F16, kind="ExternalInput")
    eprojT = dt("eprojT", [E, H], F16, kind="ExternalInput")
    valT = dt("valT", [E, H], F16, kind="ExternalInput")
    wvT = dt("wvT", [H, H], F16, kind="ExternalInput")
    aowT = dt("aowT", [H, H], F16, kind="ExternalInput")
    projb_row = dt("projb_row", [1, V], F16, kind="ExternalInput")
    biasN = dt("biasN", [128, _NBC], F32, kind="ExternalInput")
    ident_d = dt("ident", [128, 128], F16, kind="ExternalInput")
    ones1_d = dt("ones1", [1, 128], F16, kind="ExternalInput")
    out_d = dt("out", [T, BP, V], F32, kind="ExternalOutput")

    with tile.TileContext(nc) as tc:
        with (
            tc.tile_pool(name="wpool", bufs=1) as wp,
            tc.tile_pool(name="cpool", bufs=1) as cp,
            tc.tile_pool(name="h0p", bufs=2) as h0p,
            tc.tile_pool(name="h1p", bufs=2) as h1p,
            tc.tile_pool(name="gp", bufs=3) as gp,
            tc.tile_pool(name="lp", bufs=8) as lp,
            tc.tile_pool(name="psp", bufs=8, space="PSUM") as psp,
        ):
            dma = nc.sync.dma_start

            def load2d(dram, rows, cols, tag):
                """Load a (rows, cols) DRAM tensor as rows//128 SBUF k-tiles."""
                tiles = []
                for k in range(rows // 128):
                    tl = wp.tile([128, cols], F16, tag=f"{tag}{k}")
                    dma(tl[:], dram[k * 128:(k + 1) * 128, :])
                    tiles.append(tl)
                return tiles

            ew = load2d(ewordT, E, BP, "ew")
            whh0 = load2d(whh0T, H, 3 * H, "whh0")
            wih1 = load2d(wih1T, H, 3 * H, "wih1")
            whh1 = load2d(whh1T, H, 3 * H, "whh1")
            proj = load2d(projT, H, V, "proj")
            we = load2d(weT, E, 3 * H, "we")
            eproj = load2d(eprojT, E, H, "eproj")
            val = load2d(valT, E, H, "val")
            wv_t = load2d(wvT, H, H, "wv")
            aow = load2d(aowT, H, H, "aow")

            wce = wp.tile([CE, 3 * H], F16, tag="wce")
            dma(wce[:], wceT[:])
            ce_sb = cp.tile([CE, T, BP], F16, tag="ce")
            dma(ce_sb[:], ceT[:])
            pbrow = wp.tile([1, V], F16, tag="pbrow")
            dma(pbrow[:], projb_row[:])
            bias = wp.tile([128, _NBC], F32, tag="bias")
            dma(bias[:], biasN[:])
            ident = wp.tile([128, 128], F16, tag="ident")
            dma(ident[:], ident_d[:])
            ones1 = wp.tile([1, 128], F16, tag="ones1")
            dma(ones1[:], ones1_d[:])

            def bcol(c):
                return bias[:, c:c + 1]

            mm = nc.tensor.matmul
            act = nc.scalar.activation

            # ---- precompute: g_e = eword @ w_ih0[:,CE:].T + biases ----
            ge = []
            for m in range(6):
                ps = psp.tile([128, BP], F32, tag="ps")
                for k in range(6):
                    mm(ps[:], we[k][:, m * 128:(m + 1) * 128], ew[k][:],
                       start=(k == 0), stop=(k == 5))
                g = cp.tile([128, BP], F16, tag=f"ge{m}")
                act(g[:], ps[:], AF.Identity, bias=bcol(_BC_GE + m))
                ge.append(g)

            # ---- h0 = tanh(eword @ eword_proj_w.T + b), also init h1 ----
            h0_prev, h1_prev = [], []
            for m in range(2):
                ps = psp.tile([128, BP], F32, tag="ps")
                for k in range(6):
                    mm(ps[:], eproj[k][:, m * 128:(m + 1) * 128], ew[k][:],
                       start=(k == 0), stop=(k == 5))
                t0 = h0p.tile([128, BP], F16, tag=f"h0_{m}")
                t1 = h1p.tile([128, BP], F16, tag=f"h1_{m}")
                act(t0[:], ps[:], AF.Tanh, bias=bcol(_BC_H0 + m))
                act(t1[:], ps[:], AF.Tanh, bias=bcol(_BC_H0 + m))
                h0_prev.append(t0)
                h1_prev.append(t1)

            # ---- attention constant c = ao @ proj_w.T + proj_b ----
            v1 = []
            for m in range(2):
                ps = psp.tile([128, BP], F32, tag="ps")
                for k in range(6):
                    mm(ps[:], val[k][:, m * 128:(m + 1) * 128], ew[k][:],
                       start=(k == 0), stop=(k == 5))
                tl = gp.tile([128, BP], F16, tag=f"v1_{m}")
                act(tl[:], ps[:], AF.Identity, bias=bcol(_BC_VAL + m))
                v1.append(tl)
            ev = []
            for m in range(2):
                ps = psp.tile([128, BP], F32, tag="ps")
                for k in range(2):
                    mm(ps[:], wv_t[k][:, m * 128:(m + 1) * 128], v1[k][:],
                       start=(k == 0), stop=(k == 1))
                tl = gp.tile([128, BP], F16, tag=f"ev_{m}")
                act(tl[:], ps[:], AF.Identity, bias=bcol(_BC_BV + m))
                ev.append(tl)
            ao = []
            for m in range(2):
                ps = psp.tile([128, BP], F32, tag="ps")
                for k in range(2):
                    mm(ps[:], aow[k][:, m * 128:(m + 1) * 128], ev[k][:],
                       start=(k == 0), stop=(k == 1))
                tl = gp.tile([128, BP], F16, tag=f"ao_{m}")
                act(tl[:], ps[:], AF.Identity, bias=bcol(_BC_AO + m))
                ao.append(tl)
            cc = []
            for mb in range(4):
                ps = psp.tile([128, V], F32, tag="ps")
                for k in range(2):
                    mm(ps[:], ao[k][:, mb * 128:(mb + 1) * 128], proj[k][:],
                       start=(k == 0), stop=False)
                mm(ps[:], ones1[:], pbrow[:], start=False, stop=True)
                tl = cp.tile([128, V], F32, tag=f"cc{mb}")
                act(tl[:], ps[:], AF.Copy)
                cc.append(tl)

            # ---- the scan ----
            def gru_layer(h_prev, x_mm_emitter, sig_bias, hn_bias, tanh_bias,
                          hp_pool, hp_tag):
                """Emit one GRU layer's matmuls + gate math; returns new h tiles.

                x_mm_emitter(ps, g, first) emits the input-side matmuls for
                gate row-tile g into psum ps; `first` says whether it must
                start the accumulation group.
                """
                ps_rz, ps_in, ps_hn = [], [], []
                for g in range(4):  # r, z row-tiles: h-side first (its
                    # operand h_prev is ready long before the input side's)
                    ps = psp.tile([128, BP], F32, tag="ps")
                    mm(ps[:], whh_cur[0][:, g * 128:(g + 1) * 128], h_prev[0][:],
                       start=True, stop=False)
                    mm(ps[:], whh_cur[1][:, g * 128:(g + 1) * 128], h_prev[1][:],
                       start=False, stop=False)
                    x_mm_emitter(ps, g, False, True)
                    ps_rz.append(ps)
                for g in range(4, 6):  # n row-tiles: input-side alone
                    ps = psp.tile([128, BP], F32, tag="ps")
                    x_mm_emitter(ps, g, True, True)
                    ps_in.append(ps)
                for g in range(4, 6):  # n row-tiles: h-side alone
                    ps = psp.tile([128, BP], F32, tag="ps")
                    mm(ps[:], whh_cur[0][:, g * 128:(g + 1) * 128], h_prev[0][:],
                       start=True, stop=False)
                    mm(ps[:], whh_cur[1][:, g * 128:(g + 1) * 128], h_prev[1][:],
                       start=False, stop=True)
                    ps_hn.append(ps)

                h_new = []
                for i in range(2):
                    r_s = gp.tile([128, BP], F16, tag=f"r{i}")
                    act(r_s[:], ps_rz[i][:], AF.Sigmoid, bias=sig_bias(i))
                    z_s = gp.tile([128, BP], F16, tag=f"z{i}")
                    act(z_s[:], ps_rz[2 + i][:], AF.Sigmoid, bias=sig_bias(2 + i))
                    hn_s = gp.tile([128, BP], F16, tag=f"hn{i}")
                    act(hn_s[:], ps_hn[i][:], AF.Identity, bias=hn_bias(i))
                    a = gp.tile([128, BP], F16, tag=f"a{i}")
                    nc.vector.tensor_mul(a[:], r_s[:], hn_s[:])
                    u = gp.tile([128, BP], F16, tag=f"u{i}")
                    nc.vector.tensor_add(u[:], a[:], ps_in[i][:])
                    n_s = gp.tile([128, BP], F16, tag=f"n{i}")
                    act(n_s[:], u[:], AF.Tanh, bias=tanh_bias(i))
                    d = gp.tile([128, BP], F16, tag=f"d{i}")
                    nc.vector.tensor_sub(d[:], h_prev[i][:], n_s[:])
                    e2 = gp.tile([128, BP], F16, tag=f"e{i}")
                    nc.vector.tensor_mul(e2[:], d[:], z_s[:])
                    hn_t = hp_pool.tile([128, BP], F16, tag=f"{hp_tag}{i}")
                    nc.vector.tensor_add(hn_t[:], e2[:], n_s[:])
                    h_new.append(hn_t)
                return h_new

            zero = lambda _i: 0.0

            def emit_logits(h1_tiles, t):
                for mb in range(4):
                    ps = psp.tile([128, V], F32, tag="ps")
                    for k in range(2):
                        mm(ps[:], h1_tiles[k][:, mb * 128:(mb + 1) * 128],
                           proj[k][:], start=(k == 0), stop=(k == 1))
                    lo = lp.tile([128, V], F32, tag="lo")
                    nc.vector.tensor_add(lo[:], ps[:], cc[mb][:])
                    dma(out_d[t, mb * 128:(mb + 1) * 128, :], lo[:])

            prev_h1_for_logits = None
            for t in range(T):
                ce_t = ce_sb[:, t, :]

                def x0(ps, g, first, last, _ce=ce_t):
                    mm(ps[:], ident[:], ge[g][:], start=first, stop=False)
                    mm(ps[:], wce[:, g * 128:(g + 1) * 128], _ce,
                       start=False, stop=last)  # first=False inside rz groups

                whh_cur = whh0
                h0_new = gru_layer(
                    h0_prev, x0,
                    sig_bias=zero,
                    hn_bias=lambda i: bcol(_BC_HN0 + i),
                    tanh_bias=zero,
                    hp_pool=h0p, hp_tag="h0_")

                # logits of the previous step slot in here: PE has idle time
                # while layer-0 gates drain
                if prev_h1_for_logits is not None:
                    emit_logits(prev_h1_for_logits, t - 1)

                def x1(ps, g, first, last, _h0=h0_new):
                    mm(ps[:], wih1[0][:, g * 128:(g + 1) * 128], _h0[0][:],
                       start=first, stop=False)
                    mm(ps[:], wih1[1][:, g * 128:(g + 1) * 128], _h0[1][:],
                       start=False, stop=last)

                whh_cur = whh1
                h1_new = gru_layer(
                    h1_prev, x1,
                    sig_bias=lambda g: bcol(_BC_RZ1 + g),
                    hn_bias=lambda i: bcol(_BC_HN1 + i),
                    tanh_bias=lambda i: bcol(_BC_IN1 + i),
                    hp_pool=h1p, hp_tag="h1_")

                h0_prev, h1_prev = h0_new, h1_new
                prev_h1_for_logits = h1_new

            emit_logits(prev_h1_for_logits, T - 1)

    nc.compile()
    return nc


def kernel(**inputs):
    eword = np.ascontiguousarray(inputs["eword"], dtype=np.float32)
    target_ids = np.asarray(inputs["target_ids"])
    char_emb = np.asarray(inputs["char_emb"], dtype=np.float32)
    w_ih0 = np.asarray(inputs["gru_w_ih0"], dtype=np.float32)
    w_hh0 = np.asarray(inputs["gru_w_hh0"], dtype=np.float32)
    b_ih0 = np.asarray(inputs["gru_b_ih0"], dtype=np.float32)
    b_hh0 = np.asarray(inputs["gru_b_hh0"], dtype=np.float32)
    w_ih1 = np.asarray(inputs["gru_w_ih1"], dtype=np.float32)
    w_hh1 = np.asarray(inputs["gru_w_hh1"], dtype=np.float32)
    b_ih1 = np.asarray(inputs["gru_b_ih1"], dtype=np.float32)
    b_hh1 = np.asarray(inputs["gru_b_hh1"], dtype=np.float32)
    attn_in_w = np.asarray(inputs["attn_in_w"], dtype=np.float32)
    attn_in_b = np.asarray(inputs["attn_in_b"], dtype=np.float32)
    attn_out_w = np.asarray(inputs["attn_out_w"], dtype=np.float32)
    attn_out_b = np.asarray(inputs["attn_out_b"], dtype=np.float32)
    eword_proj_w = np.asarray(inputs["eword_proj_w"], dtype=np.float32)
    eword_proj_b = np.asarray(inputs["eword_proj_b"], dtype=np.float32)
    val_w = np.asarray(inputs["val_w"], dtype=np.float32)
    val_b = np.asarray(inputs["val_b"], dtype=np.float32)
    proj_w = np.asarray(inputs["proj_w"], dtype=np.float32)
    proj_b = np.asarray(inputs["proj_b"], dtype=np.float32)

    f16 = np.float16

    in_ids = np.concatenate(
        [np.full((B, 1), BOS, target_ids.dtype), target_ids[:, :-1]], axis=1)
    ce = char_emb[in_ids] * (in_ids != PAD)[..., None].astype(np.float32)

    wv = attn_in_w[2 * H:3 * H]
    bv = attn_in_b[2 * H:3 * H]

    shared = {
        "whh0T": np.ascontiguousarray(w_hh0.T, dtype=f16),
        "wceT": np.ascontiguousarray(w_ih0[:, :CE].T, dtype=f16),
        "wih1T": np.ascontiguousarray(w_ih1.T, dtype=f16),
        "whh1T": np.ascontiguousarray(w_hh1.T, dtype=f16),
        "projT": np.ascontiguousarray(proj_w.T, dtype=f16),
        "weT": np.ascontiguousarray(w_ih0[:, CE:].T, dtype=f16),
        "eprojT": np.ascontiguousarray(eword_proj_w.T, dtype=f16),
        "valT": np.ascontiguousarray(val_w.T, dtype=f16),
        "wvT": np.ascontiguousarray(wv.T, dtype=f16),
        "aowT": np.ascontiguousarray(attn_out_w.T, dtype=f16),
        "projb_row": np.ascontiguousarray(proj_b[None, :], dtype=f16),
        "ident": np.eye(128, dtype=f16),
        "ones1": np.ones((1, 128), dtype=f16),
    }

    bias = np.zeros((128, _NBC), np.float32)
    for m in range(6):
        col = b_ih0[m * 128:(m + 1) * 128].copy()
        if m < 4:
            col += b_hh0[m * 128:(m + 1) * 128]
        bias[:, _BC_GE + m] = col
    for m in range(2):
        bias[:, _BC_H0 + m] = eword_proj_b[m * 128:(m + 1) * 128]
        bias[:, _BC_VAL + m] = val_b[m * 128:(m + 1) * 128]
        bias[:, _BC_BV + m] = bv[m * 128:(m + 1) * 128]
        bias[:, _BC_AO + m] = attn_out_b[m * 128:(m + 1) * 128]
        bias[:, _BC_HN0 + m] = b_hh0[2 * H + m * 128:2 * H + (m + 1) * 128]
        bias[:, _BC_HN1 + m] = b_hh1[2 * H + m * 128:2 * H + (m + 1) * 128]
        bias[:, _BC_IN1 + m] = b_ih1[2 * H + m * 128:2 * H + (m + 1) * 128]
    b1 = b_ih1 + b_hh1
    for m in range(4):
        bias[:, _BC_RZ1 + m] = b1[m * 128:(m + 1) * 128]
    shared["biasN"] = bias

    in_maps = []
    for c in range(NCORES):
        sl = slice(c * BP, (c + 1) * BP)
        m = dict(shared)
        m["ewordT"] = np.ascontiguousarray(eword[sl].T, dtype=f16)
        m["ceT"] = np.ascontiguousarray(
            ce[sl].transpose(2, 1, 0), dtype=f16)  # (CE, T, BP)
        in_maps.append(m)

    if "nc" not in _CACHE:
        _CACHE["nc"] = _build_nc()
    nc = _CACHE["nc"]

    res = run_bass_kernel_spmd(nc, in_maps, list(range(NCORES)),
                               trace=bool(os.environ.get("BASS_TRACE")))
    _CACHE["last_res"] = res
    outs = [res.results[c]["out"].transpose(1, 0, 2) for c in range(NCORES)]
    return np.ascontiguousarray(np.concatenate(outs, axis=0))

